# revision 26
# baseline (speedup 1.0000x reference)
"""ConvNeXt block (dwconv7 -> LN -> pwconv1 -> GELU -> GRN -> pwconv2 -> residual)
for Trainium2, batch-parallel across 8 NeuronCores (2 batches per core).

Self-contained: hardcodes shapes B=16, C=512, T=2048, I=1536, K=7.

v2 design (vs v1 baseline):
  - h kept entirely in SBUF (f16 tiles, 5-deep pool) -- no DRAM spill.
  - LN mean handled by a second broadcast matmul (bbc = -mu) + in-place
    pool add, replacing the per-ii rank-1 W1s matmuls.
  - 8 global t-tiles (2 batches x 4); per-tile work balanced across
    DVE/Pool/Act so each engine carries ~12us per 12.4us PE block:
      DVE:  conv taps {1,3,5}, GRN squares ii 0..5, row chain
      Pool: conv taps {0,2,4,6}, ysq, LN apply, w2 scaling, residual add
      Act:  GELU, GRN squares ii 6..11, stdv
  - mm2 of batch 0 interleaves with mm1 of batch 1; GRN chains hide under
    matmuls; conv is split across block boundaries for early LN stats.
  - All DMA issued from SP/Act HWDGE queues (none from Pool/SWDGE).
  - w2 scaled into a separate buffer for b0, in place for b1 (single load).

Math folding (host-side, weight-sized only):
  LN:  y_ln = (y - mu_t) * A_t * ln_g + ln_b      (A_t = rsqrt(var_t + eps))
  mm1: h = GELU( w1p @ ((y - mu)*A) + b1p ),  w1p = w1*ln_g, b1p = b1 + w1@ln_b
  GRN: h' = h * ss[i] + grn_b[i],  ss = 1 + grn_g * gx * d,
       gx = sqrt(sum_t h^2), d = 1/(mean_i gx + eps)
  mm2: out = (w2 * ss).T-contracted with h + (b2 + w2 @ grn_b) + residual
"""
import sys

sys.path.insert(0, "/opt/trn_rl_repo")

import numpy as np
import concourse.bacc as bacc
import concourse.tile as tile
from concourse import mybir
from concourse.bass_utils import run_bass_kernel_spmd

F32 = mybir.dt.float32
F32R = mybir.dt.float32r
F16 = mybir.dt.float16
BF16 = mybir.dt.bfloat16
AF = mybir.ActivationFunctionType
OP = mybir.AluOpType

B, C, T, I, K = 16, 512, 2048, 1536, 7
NCORES = 8
BPC = B // NCORES          # batches per core
CC = C // 128              # 4 c-chunks
IC = I // 128              # 12 i-chunks
TC = T // 512              # 4 t-chunks per batch
TN = 512                   # matmul free-dim tile
G = BPC * TC               # 8 global tiles
LN_EPS = 1e-6
GRN_EPS = 1e-6

_CACHE = {}


def _build(trace_sim=False, reps=1):
    nc = bacc.Bacc("TRN2", target_bir_lowering=False, debug=False,
                   num_devices=NCORES)
    dram = {}

    def din(name, shape, dt=F32):
        dram[name] = nc.dram_tensor(name, shape, dt, kind="ExternalInput").ap()
        return dram[name]

    x_d = din("x", [BPC, C, T])                      # per-core batches
    w1pT_d = din("w1pT", [C, I], F32R)               # (w1*ln_g).T  [c, i]
    b1p_d = din("b1p", [128, IC])                    # col-chunked b1p
    w2T_d = din("w2T", [I, C], F16)                  # w2.T  [i, c]
    b2p_d = din("b2p", [128, CC])                    # col-chunked b2 + w2@grn_b
    grng_d = din("grng", [128, IC])                  # col-chunked grn_g
    dww_d = din("dww", [128, CC, K])                 # depthwise taps per c-chunk
    dwb_d = din("dwb", [128, CC])                    # depthwise bias per c-chunk
    out_d = nc.dram_tensor("out", [BPC, C, T], F32, kind="ExternalOutput").ap()

    with tile.TileContext(nc, trace_sim=trace_sim) as tc:
        for _ in range(reps):
            _kernel_body(nc, tc, x_d, w1pT_d, b1p_d, w2T_d, b2p_d,
                         grng_d, dww_d, dwb_d, out_d)
    nc.compile()
    return nc


def _kernel_body(nc, tc, x_d, w1pT_d, b1p_d, w2T_d, b2p_d,
                 grng_d, dww_d, dwb_d, out_d):
    from contextlib import ExitStack
    XH = TN + 6
    ctx = ExitStack()
    with ctx:
        ctx.enter_context(nc.allow_low_precision(
            reason="f32r matmul operand rounding is intentional"))
        singles = ctx.enter_context(tc.tile_pool(name="singles", bufs=1))
        xhp = ctx.enter_context(tc.tile_pool(name="xhp", bufs=10))
        xrp = ctx.enter_context(tc.tile_pool(name="xrp", bufs=4))
        yp = ctx.enter_context(tc.tile_pool(name="yp", bufs=2))
        ysqp = ctx.enter_context(tc.tile_pool(name="ysqp", bufs=1))
        yscp = ctx.enter_context(tc.tile_pool(name="yscp", bufs=2))
        hp = ctx.enter_context(tc.tile_pool(name="hp", bufs=5))
        rowp = ctx.enter_context(tc.tile_pool(name="rowp", bufs=3))
        rowmp = ctx.enter_context(tc.tile_pool(name="rowmp", bufs=2))
        rowap = ctx.enter_context(tc.tile_pool(name="rowap", bufs=2))
        sqp = ctx.enter_context(tc.tile_pool(name="sqp", bufs=1))
        gxp = ctx.enter_context(tc.tile_pool(name="gxp", bufs=2))
        w2sp = ctx.enter_context(tc.tile_pool(name="w2sp", bufs=1))
        bcsb = ctx.enter_context(tc.tile_pool(name="bcsb", bufs=2))
        cvp = ctx.enter_context(tc.tile_pool(name="cvp", bufs=1))
        obp = ctx.enter_context(tc.tile_pool(name="obp", bufs=2))
        # PSUM: 4 matmul banks + 2 stats banks + 2 broadcast banks = 8
        mmps = ctx.enter_context(tc.tile_pool(name="mmps", bufs=4, space="PSUM"))
        smps = ctx.enter_context(tc.tile_pool(name="smps", bufs=2, space="PSUM"))
        bcps = ctx.enter_context(tc.tile_pool(name="bcps", bufs=2, space="PSUM"))

        # ---- constants (small ones first so conv starts immediately) ----
        dww = singles.tile([128, CC, K], F32)
        nc.sync.dma_start(dww[:], dww_d)
        dwb = singles.tile([128, CC], F32)
        nc.sync.dma_start(dwb[:], dwb_d)

        xv = x_d.rearrange("b (cc p) t -> b p cc t", p=128)
        ov = out_d.rearrange("b (cc p) t -> b p cc t", p=128)

        # state carried between emission steps
        xh_t = [None] * G          # conv x tiles (with halo)
        xh_meta = [None] * G       # (lo, off, xn)
        y_t = [None] * G
        ysq_t = [None] * G
        ysc_t = [None] * G
        h_t = [None] * G
        row_t = [None] * G         # [M_row, A_row]
        sm_t = [None] * G          # [sumy, sumsq]
        bc_t = [None] * G          # [bbc_ps, abc_ps]
        xr_t = [None] * G
        gxpart = [None] * BPC
        ss_t = [None] * BPC
        w2x = [None] * BPC         # scaled w2 for mm2 (lhsT)

        def emit_x_dma(g):
            bb, tt = divmod(g, TC)
            t0 = tt * TN
            lo = max(0, t0 - 3)
            hi = min(T, t0 + TN + 3)
            xn = hi - lo
            xhs = []
            for ci in range(CC):
                xh = xhp.tile([128, xn], F32, tag="xh", name="xh",
                              padded_shape=[128, XH])
                nc.sync.dma_start(xh[:], xv[bb, :, ci, lo:hi])
                xhs.append(xh)
            xh_t[g] = xhs
            xh_meta[g] = (lo, t0 - lo, xn)

        def emit_conv(g, cis):
            # conv taps: DVE {1,3(center+bias),5}; Pool {0,2,4,6}; then
            # ysq = y*y per-ci on Pool (feeds the sumsq stats matmuls).
            bb, tt = divmod(g, TC)
            t0 = tt * TN
            lo, off, xn = xh_meta[g]
            if y_t[g] is None:
                y_t[g] = yp.tile([128, CC, TN], F32R, tag="y", name="y")
                ysq_t[g] = ysqp.tile([128, CC, TN], BF16, tag="ysq", name="ysq")
            y = y_t[g]
            ysq = ysq_t[g]
            def rng(k):
                d = k - 3
                jlo = max(0, -(t0 + d))
                jhi = min(TN, T - (t0 + d))
                return d, jlo, jhi, off + d + jlo
            for ci in cis:
                acc = y[:, ci, :]
                xh = xh_t[g][ci]
                # center tap + bias initializes the accumulator (Pool ts2)
                nc.gpsimd.tensor_scalar(acc, xh[:, off:off + TN],
                                        dww[:, ci, 3:4], dwb[:, ci:ci + 1],
                                        OP.mult, OP.add)
                # stt taps on DVE accumulate in place
                for k in (1, 2, 5):
                    d, jlo, jhi, s0 = rng(k)
                    nc.vector.scalar_tensor_tensor(
                        acc[:, jlo:jhi], xh[:, s0:s0 + (jhi - jlo)],
                        dww[:, ci, k:k + 1], acc[:, jlo:jhi],
                        OP.mult, OP.add)
                # product taps: Pool {0,6} and Act {4}, then adds
                d, jlo0, jhi0, s00 = rng(0)
                p0 = cvp.tile([128, TN], F32, tag="p0", name="p0")
                nc.gpsimd.tensor_scalar(p0[:, jlo0:jhi0],
                                        xh[:, s00:s00 + (jhi0 - jlo0)],
                                        dww[:, ci, 0:1], None, OP.mult)
                d, jlo6, jhi6, s06 = rng(6)
                p6 = cvp.tile([128, TN], F32, tag="p6", name="p6")
                nc.gpsimd.tensor_scalar(p6[:, jlo6:jhi6],
                                        xh[:, s06:s06 + (jhi6 - jlo6)],
                                        dww[:, ci, 6:7], None, OP.mult)
                d, jlo4, jhi4, s04 = rng(4)
                p4 = cvp.tile([128, TN], F32, tag="p4", name="p4")
                nc.scalar.activation(p4[:, jlo4:jhi4],
                                     xh[:, s04:s04 + (jhi4 - jlo4)],
                                     AF.Copy, scale=dww[:, ci, 4:5])
                nc.gpsimd.tensor_add(acc[:, jlo0:jhi0], acc[:, jlo0:jhi0],
                                     p0[:, jlo0:jhi0])
                nc.gpsimd.tensor_add(acc[:, jlo6:jhi6], acc[:, jlo6:jhi6],
                                     p6[:, jlo6:jhi6])
                nc.vector.tensor_add(acc[:, jlo4:jhi4], acc[:, jlo4:jhi4],
                                     p4[:, jlo4:jhi4])
                nc.gpsimd.tensor_mul(ysq[:, ci, :], acc.bitcast(F32),
                                     acc.bitcast(F32))

        def emit_stats_a(g):
            y = y_t[g]
            ysq = ysq_t[g]
            sumy = smps.tile([1, TN], F32, tag="sm", name="sumy")
            sumsq = smps.tile([1, TN], F32, tag="sm", name="sumsq")
            for ci in (0, 1):
                nc.tensor.matmul(sumy[:], ones_col[:], y[:, ci, :],
                                 start=(ci == 0), stop=False)
                nc.tensor.matmul(sumsq[:], ones_col_bf[:], ysq[:, ci, :],
                                 start=(ci == 0), stop=False)
            sm_t[g] = [sumy, sumsq]

        def emit_sumy(g):
            y = y_t[g]
            sumy = sm_t[g][0]
            for ci in (2, 3):
                nc.tensor.matmul(sumy[:], ones_col[:], y[:, ci, :],
                                 start=False, stop=(ci == CC - 1))
            # M_row = -mu; msq/meps gated only on sumy -> run early on Pool
            M_row = rowmp.tile([1, TN], F32R, tag="rowm", name="mrow")
            nc.vector.tensor_scalar(M_row[:], sumy[:], -1.0 / C, None, OP.mult)
            msq = rowp.tile([1, TN], F32, tag="row", name="msq")
            nc.gpsimd.tensor_mul(msq[:], M_row[:].bitcast(F32),
                                 M_row[:].bitcast(F32))
            meps = rowp.tile([1, TN], F32, tag="row", name="meps")
            nc.gpsimd.tensor_scalar(meps[:], msq[:], LN_EPS, None, OP.subtract)
            row_t[g] = [M_row, None, meps]

        def emit_sumsq(g):
            ysq = ysq_t[g]
            sumsq = sm_t[g][1]
            for ci in (2, 3):
                nc.tensor.matmul(sumsq[:], ones_col_bf[:], ysq[:, ci, :],
                                 start=False, stop=(ci == CC - 1))
            meps = row_t[g][2]
            vpe = rowp.tile([1, TN], F32, tag="row", name="vpe")
            nc.vector.scalar_tensor_tensor(vpe[:], sumsq[:], 1.0 / C, meps[:],
                                           OP.mult, OP.subtract)
            # A = rsqrt(vpe): bit-trick seed + 1 Newton step (no act table)
            sh = rowap.tile([1, TN], U32, tag="rowa", name="sh")
            nc.vector.tensor_scalar(sh[:], vpe[:].bitcast(U32), one_row[:],
                                    None, OP.logical_shift_right)
            a0 = rowap.tile([1, TN], U32, tag="rowa", name="a0")
            nc.vector.scalar_tensor_tensor(a0[:], magic_row[:], 1.0, sh[:],
                                           OP.bypass, OP.subtract)
            t1 = rowp.tile([1, TN], F32, tag="row", name="t1")
            nc.gpsimd.tensor_mul(t1[:], a0[:].bitcast(F32), a0[:].bitcast(F32))
            t2 = rowp.tile([1, TN], F32, tag="row", name="t2")
            nc.gpsimd.tensor_mul(t2[:], t1[:], vpe[:])
            t3 = rowp.tile([1, TN], F32, tag="row", name="t3")
            nc.vector.tensor_scalar(t3[:], t2[:], -0.5, 1.5, OP.mult, OP.add)
            A_row = rowp.tile([1, TN], F32R, tag="row", name="arow")
            nc.vector.tensor_mul(A_row[:], a0[:].bitcast(F32), t3[:])
            row_t[g][1] = A_row

        def emit_bbc(g):
            M_row = row_t[g][0]
            bbc_ps = bcps.tile([128, TN], F32, tag="bc", name="bbcps")
            nc.tensor.matmul(bbc_ps[:], ones_row[:], M_row[:],
                             start=True, stop=True)
            bbc = bcsb.tile([128, TN], F32, tag="bcs", name="bbc")
            nc.scalar.activation(bbc[:], bbc_ps[:], AF.Copy)
            bc_t[g] = [bbc, None]
            # y += bbc (in place on Pool, SBUF only)
            y = y_t[g]
            for ci in range(CC):
                nc.gpsimd.tensor_add(y[:, ci, :], y[:, ci, :].bitcast(F32),
                                     bbc[:])

        def emit_abc(g):
            A_row = row_t[g][1]
            abc_ps = bcps.tile([128, TN], F32, tag="bc", name="abcps")
            nc.tensor.matmul(abc_ps[:], ones_row[:], A_row[:],
                             start=True, stop=True)
            abc = bcsb.tile([128, TN], F32, tag="bcs", name="abc")
            nc.vector.tensor_copy(abc[:], abc_ps[:])
            bc_t[g][1] = abc
            # ysc = y * abc  (pool reads PSUM)
            y = y_t[g]
            ysc = yscp.tile([128, CC, TN], F32R, tag="ysc", name="ysc")
            for ci in range(CC):
                nc.gpsimd.tensor_mul(ysc[:, ci, :], y[:, ci, :].bitcast(F32),
                                     abc[:])
            ysc_t[g] = ysc

        def emit_mm1(g, interject=None):
            bb, tt = divmod(g, TC)
            ysc = ysc_t[g]
            h = hp.tile([128, IC, TN], F16, tag="h", name="h")
            h_t[g] = h
            for ii in range(IC):
                if interject is not None:
                    interject(ii)
                ph = mmps.tile([128, TN], F32, tag="mm", name="ph")
                isl = slice(ii * 128, (ii + 1) * 128)
                for ci in range(CC):
                    nc.tensor.matmul(ph[:], w1pT[:, ci, isl], ysc[:, ci, :],
                                     start=(ci == 0), stop=(ci == CC - 1))
                nc.scalar.activation(h[:, ii, :], ph[:], AF.Gelu,
                                     bias=b1p[:, ii:ii + 1])
                # GRN partial: sum_t h^2 for this (ii, t); DVE/Act split
                ndve = 5
                if ii < ndve:
                    sqd = sqp.tile([128, TN], F16, tag="sq", name="sqd")
                    nc.vector.scalar_tensor_tensor(
                        sqd[:], h[:, ii, :], 1.0, h[:, ii, :],
                        OP.bypass, OP.mult,
                        accum_out=gxpart[bb][:, ii, tt:tt + 1])
                else:
                    sqd = sqp.tile([128, TN], F16, tag="sqa", name="sqa")
                    nc.scalar.activation(
                        sqd[:], h[:, ii, :], AF.Square,
                        accum_out=gxpart[bb][:, ii, tt:tt + 1])

        def emit_grn(bb):
            gxsq = gxp.tile([128, IC], F32, tag="gx2", name="gxsq")
            nc.vector.tensor_reduce(gxsq[:], gxpart[bb][:],
                                    axis=mybir.AxisListType.X, op=OP.add)
            gsh = gxp.tile([128, IC], U32, tag="gxa", name="gsh")
            nc.vector.tensor_scalar(gsh[:], gxsq[:].bitcast(U32), one_col[:],
                                    None, OP.logical_shift_right)
            ga0 = gxp.tile([128, IC], U32, tag="gxa", name="ga0")
            nc.vector.scalar_tensor_tensor(ga0[:], magic_col[:], 1.0, gsh[:],
                                           OP.bypass, OP.subtract)
            gt1 = gxp.tile([128, IC], F32, tag="gxn", name="gt1")
            nc.vector.tensor_mul(gt1[:], ga0[:].bitcast(F32),
                                 ga0[:].bitcast(F32))
            gt2 = gxp.tile([128, IC], F32, tag="gxn", name="gt2")
            nc.vector.tensor_mul(gt2[:], gt1[:], gxsq[:])
            gt3 = gxp.tile([128, IC], F32, tag="gxn", name="gt3")
            nc.vector.tensor_scalar(gt3[:], gt2[:], -0.5, 1.5, OP.mult, OP.add)
            grs = gxp.tile([128, IC], F32, tag="gxn", name="grs")
            nc.vector.tensor_mul(grs[:], ga0[:].bitcast(F32), gt3[:])
            gx = gxp.tile([128, IC], F32R, tag="gx2", name="gx")
            nc.vector.tensor_mul(gx[:], gxsq[:], grs[:])
            gsum = smps.tile([1, IC], F32, tag="sm", name="gsum")
            nc.tensor.matmul(gsum[:], ones_col[:], gx[:], start=True, stop=True)
            gtot = gxp.tile([1, 1], F32, tag="gx3", name="gtot")
            nc.vector.tensor_reduce(gtot[:], gsum[:],
                                    axis=mybir.AxisListType.X, op=OP.add)
            dinv = gxp.tile([1, 1], F32, tag="gx3", name="dinv")
            nc.vector.tensor_scalar(dinv[:], gtot[:], 1.0 / I, GRN_EPS,
                                    OP.mult, OP.add)
            d_row = gxp.tile([1, 1], F32R, tag="gx3", name="drow")
            nc.vector.reciprocal(d_row[:], dinv[:])
            dbc = gxp.tile([128, 1], F32, tag="gx4", name="dbc")
            nc.gpsimd.partition_broadcast(dbc[:], d_row[:].bitcast(F32))
            ss = gxp.tile([128, IC], F32, tag="gx4", name="ss")
            nc.vector.scalar_tensor_tensor(ss[:], gx[:].bitcast(F32), dbc[:],
                                           grng[:], OP.mult, OP.mult)
            nc.vector.tensor_scalar(ss[:], ss[:], 1.0, None, OP.add)
            ss_t[bb] = ss

        def emit_w2scale(bb, iis):
            # w2 * ss, alternating Pool/DVE so mm2 is fed quickly;
            # b0 into a fresh buffer, b1 in place
            ss = ss_t[bb]
            if bb == 0 and w2x[0] is None:
                w2x[0] = w2sp.tile([128, IC, C], F16, tag="w2s", name="w2s")
            dst = w2x[0] if bb == 0 else w2t
            for ii in iis:
                eng = nc.gpsimd if ii % 2 == 0 else nc.vector
                eng.tensor_scalar(dst[:, ii, :], w2t[:, ii, :],
                                  ss[:, ii:ii + 1], None, OP.mult)
            if bb == 1:
                w2x[1] = w2t

        def emit_xres_dma(g):
            bb, tt = divmod(g, TC)
            xrs = []
            for ci in range(CC):
                xr = xrp.tile([128, TN], F32, tag="xr", name="xr")
                nc.sync.dma_start(xr[:], xv[bb, :, ci, tt * TN:(tt + 1) * TN])
                xrs.append(xr)
            xr_t[g] = xrs

        def emit_mm2(g):
            bb, tt = divmod(g, TC)
            h = h_t[g]
            w2 = w2x[bb]
            for ci in range(CC):
                po = mmps.tile([128, TN], F32, tag="mm", name="po")
                csl = slice(ci * 128, (ci + 1) * 128)
                for ii in range(IC):
                    nc.tensor.matmul(po[:], w2[:, ii, csl], h[:, ii, :],
                                     start=(ii == 0), stop=(ii == IC - 1))
                # drain per ci: (po + b2p) + x_res fused on DVE
                o_sb = obp.tile([128, TN], F32, tag="ob", name="osb")
                nc.vector.scalar_tensor_tensor(
                    o_sb[:], po[:], b2p[:, ci:ci + 1], xr_t[g][ci][:],
                    OP.add, OP.add)
                nc.sync.dma_start(ov[bb][:, ci, tt * TN:(tt + 1) * TN],
                                  o_sb[:])

        # ---------------- emission schedule ----------------
        emit_x_dma(0)
        emit_x_dma(1)

        # weights after the first x tiles so conv starts promptly
        w1pT = singles.tile([128, CC, I], F32R)
        nc.sync.dma_start(w1pT[:], w1pT_d.rearrange("(cc p) i -> p cc i", p=128))
        b1p = singles.tile([128, IC], F32)
        nc.sync.dma_start(b1p[:], b1p_d)
        b2p = singles.tile([128, CC], F32)
        nc.sync.dma_start(b2p[:], b2p_d)
        grng = singles.tile([128, IC], F32)
        nc.sync.dma_start(grng[:], grng_d)
        w2t = singles.tile([128, IC, C], F16)
        nc.scalar.dma_start(w2t[:], w2T_d.rearrange("(ic p) c -> p ic c", p=128))

        onesf = singles.tile([128, 1], F32)
        nc.vector.memset(onesf[:], 1.0)
        ones_col = singles.tile([128, 1], F32R)   # stats lhsT (K=128, M=1)
        nc.vector.tensor_copy(ones_col[:], onesf[:])
        onesrf = singles.tile([1, 128], F32)
        nc.vector.memset(onesrf[:], 1.0)
        ones_row = singles.tile([1, 128], F32R)   # bcast lhsT (K=1, M=128)
        nc.vector.tensor_copy(ones_row[:], onesrf[:])
        ones_col_bf = singles.tile([128, 1], BF16)
        nc.vector.tensor_copy(ones_col_bf[:], onesf[:])
        U32 = mybir.dt.uint32
        magic_row = singles.tile([1, TN], U32)
        nc.vector.memset(magic_row[:], 0x5F3759DF)
        one_row = singles.tile([1, 1], U32)
        nc.vector.memset(one_row[:], 1)
        magic_col = singles.tile([128, IC], U32)
        nc.vector.memset(magic_col[:], 0x5F3759DF)
        one_col = singles.tile([128, 1], U32)
        nc.vector.memset(one_col[:], 1)

        gxpart[0] = gxp.tile([128, IC, TC], F32, tag="gxpart", name="gxp0")
        gxpart[1] = gxp.tile([128, IC, TC], F32, tag="gxpart", name="gxp1")

        # tile 0 prologue (PE idles here once)
        for _ci in range(CC):
            emit_conv(0, (_ci,))
        emit_sumy(0)
        emit_sumsq(0)
        emit_bbc(0)
        emit_abc(0)
        emit_conv(1, (0, 1))   # half of tile 1's conv ahead of block 0
        emit_stats_a(1)

        for g in range(G):
            # prefetch x two tiles ahead; second half of next tile's conv
            if g + 2 < G:
                emit_x_dma(g + 2)
            if g + 1 < G:
                emit_conv(g + 1, (2, 3))
            if g >= 3 and g - 3 < TC:          # x_res for b0's mm2 tiles
                emit_xres_dma(g - 3)

            gn = g + 1 if g + 1 < G else None

            def interject(ii, gn=gn, g=g):
                if gn is not None:
                    if ii == 2:
                        emit_sumy(gn)
                    elif ii == 4:
                        emit_sumsq(gn)
                    elif ii == 6:
                        emit_bbc(gn)
                    elif ii == 8:
                        emit_abc(gn)
                if g == 4 and ii in (3, 7, 11):    # w2s for b0, in chunks
                    emit_w2scale(0, range(ii - 3, ii + 1))

            emit_mm1(g, interject)

            if g == TC - 1:
                emit_grn(0)
            if TC <= g < 2 * TC:
                emit_mm2(g - TC)
            if g + 2 < G:
                emit_conv(g + 2, (0, 1))   # first half of conv, a block early
                emit_stats_a(g + 2)

        emit_grn(1)
        emit_w2scale(1, range(IC))
        for t in range(TC):
            emit_xres_dma(TC + t)
            emit_mm2(TC + t)


def _host_prep(inputs):
    w1 = inputs["w1"].astype(np.float64)
    ln_g = inputs["ln_g"].astype(np.float64)
    ln_b = inputs["ln_b"].astype(np.float64)
    w2 = inputs["w2"].astype(np.float64)
    w1p = w1 * ln_g[None, :]                         # [I, C]
    prep = {
        "w1pT": np.ascontiguousarray(w1p.T).astype(np.float32),
        "b1p": (inputs["b1"].astype(np.float64) + w1 @ ln_b)
               .astype(np.float32).reshape(IC, 128).T.copy(),
        "w2T": np.ascontiguousarray(w2.T).astype(np.float16),
        "b2p": (inputs["b2"].astype(np.float64)
                + w2 @ inputs["grn_b"].astype(np.float64))
               .astype(np.float32).reshape(CC, 128).T.copy(),
        "grng": inputs["grn_g"].reshape(IC, 128).T.copy().astype(np.float32),
        "dww": inputs["dw_w"].reshape(C, K).reshape(CC, 128, K)
               .transpose(1, 0, 2).copy().astype(np.float32),
        "dwb": inputs["dw_b"].reshape(CC, 128).T.copy().astype(np.float32),
    }
    return prep


def run(inputs, trace=False, **kw):
    if "nc" not in _CACHE:
        _CACHE["nc"] = _build()
    nc = _CACHE["nc"]
    prep = _host_prep(inputs)
    x = np.asarray(inputs["x"], dtype=np.float32)
    in_maps = []
    for c in range(NCORES):
        m = dict(prep)
        m["x"] = np.ascontiguousarray(x[c * BPC:(c + 1) * BPC])
        in_maps.append(m)
    res = run_bass_kernel_spmd(nc, in_maps, core_ids=list(range(NCORES)),
                               trace=trace, **kw)
    out = np.concatenate([r["out"] for r in res.results], axis=0)
    return out, res


def kernel(**inputs):
    out, _ = run(inputs)
    return out


# revision 37
# speedup vs baseline: 3.3566x; 3.3566x over previous
"""ConvNeXt block (dwconv7 -> LN -> pwconv1 -> GELU -> GRN -> pwconv2 -> residual)
for Trainium2, batch-parallel across 8 NeuronCores (2 batches per core).

Self-contained: hardcodes shapes B=16, C=512, T=2048, I=1536, K=7.

v2 design (vs v1 baseline):
  - h kept entirely in SBUF (f16 tiles, 5-deep pool) -- no DRAM spill.
  - LN mean handled by a second broadcast matmul (bbc = -mu) + in-place
    pool add, replacing the per-ii rank-1 W1s matmuls.
  - 8 global t-tiles (2 batches x 4); per-tile work balanced across
    DVE/Pool/Act so each engine carries ~12us per 12.4us PE block:
      DVE:  conv taps {1,3,5}, GRN squares ii 0..5, row chain
      Pool: conv taps {0,2,4,6}, ysq, LN apply, w2 scaling, residual add
      Act:  GELU, GRN squares ii 6..11, stdv
  - mm2 of batch 0 interleaves with mm1 of batch 1; GRN chains hide under
    matmuls; conv is split across block boundaries for early LN stats.
  - All DMA issued from SP/Act HWDGE queues (none from Pool/SWDGE).
  - w2 scaled into a separate buffer for b0, in place for b1 (single load).

Math folding (host-side, weight-sized only):
  LN:  y_ln = (y - mu_t) * A_t * ln_g + ln_b      (A_t = rsqrt(var_t + eps))
  mm1: h = GELU( w1p @ ((y - mu)*A) + b1p ),  w1p = w1*ln_g, b1p = b1 + w1@ln_b
  GRN: h' = h * ss[i] + grn_b[i],  ss = 1 + grn_g * gx * d,
       gx = sqrt(sum_t h^2), d = 1/(mean_i gx + eps)
  mm2: out = (w2 * ss).T-contracted with h + (b2 + w2 @ grn_b) + residual
"""
import sys

sys.path.insert(0, "/opt/trn_rl_repo")

import numpy as np
import concourse.bacc as bacc
import concourse.tile as tile
from concourse import mybir
from concourse.bass_utils import run_bass_kernel_spmd

F32 = mybir.dt.float32
F32R = mybir.dt.float32r
F16 = mybir.dt.float16
BF16 = mybir.dt.bfloat16
U32 = mybir.dt.uint32
AF = mybir.ActivationFunctionType
OP = mybir.AluOpType

B, C, T, I, K = 16, 512, 2048, 1536, 7
NCORES = 8
BPC = B // NCORES          # batches per core
CC = C // 128              # 4 c-chunks
IC = I // 128              # 12 i-chunks
TC = T // 512              # 4 t-chunks per batch
TN = 512                   # matmul free-dim tile
G = BPC * TC               # 8 global tiles
LN_EPS = 1e-6
GRN_EPS = 1e-6

_CACHE = {}


def _build(trace_sim=False, reps=1):
    nc = bacc.Bacc("TRN2", target_bir_lowering=False, debug=False,
                   num_devices=NCORES)
    dram = {}

    def din(name, shape, dt=F32):
        dram[name] = nc.dram_tensor(name, shape, dt, kind="ExternalInput").ap()
        return dram[name]

    x_d = din("x", [BPC, C, T], F32R)                # per-core batches
    w1pT_d = din("w1pT", [C, I], F32R)               # (w1*ln_g).T  [c, i]
    b1p_d = din("b1p", [128, IC])                    # col-chunked b1p
    w2T_d = din("w2T", [I, C], F16)                  # w2.T  [i, c]
    b2p_d = din("b2p", [128, CC])                    # col-chunked b2 + w2@grn_b
    grng_d = din("grng", [128, IC])                  # col-chunked grn_g
    dww_d = din("dww", [128, CC, K])                 # depthwise taps per c-chunk
    dwdg_d = din("dwdg", [128, CC, K, 128], F32R)    # diag taps for PE conv
    dwb_d = din("dwb", [128, CC])                    # depthwise bias per c-chunk
    out_d = nc.dram_tensor("out", [BPC, C, T], F32, kind="ExternalOutput").ap()

    with tile.TileContext(nc, trace_sim=trace_sim) as tc:
        for _ in range(reps):
            _kernel_body(nc, tc, x_d, w1pT_d, b1p_d, w2T_d, b2p_d,
                         grng_d, dww_d, dwb_d, dwdg_d, out_d)
    nc.compile()
    return nc


def _kernel_body(nc, tc, x_d, w1pT_d, b1p_d, w2T_d, b2p_d,
                 grng_d, dww_d, dwb_d, dwdg_d, out_d):
    from contextlib import ExitStack
    XH = TN + 6
    ctx = ExitStack()
    with ctx:
        ctx.enter_context(nc.allow_low_precision(
            reason="f32r matmul operand rounding is intentional"))
        singles = ctx.enter_context(tc.tile_pool(name="singles", bufs=1))
        xhp = ctx.enter_context(tc.tile_pool(name="xhp", bufs=8))
        xrp = ctx.enter_context(tc.tile_pool(name="xrp", bufs=3))
        yp = ctx.enter_context(tc.tile_pool(name="yp", bufs=2))
        ysqp = ctx.enter_context(tc.tile_pool(name="ysqp", bufs=1))
        yscp = ctx.enter_context(tc.tile_pool(name="yscp", bufs=2))
        hp = ctx.enter_context(tc.tile_pool(name="hp", bufs=5))
        rowp = ctx.enter_context(tc.tile_pool(name="rowp", bufs=3))
        rowmp = ctx.enter_context(tc.tile_pool(name="rowmp", bufs=2))
        rowap = ctx.enter_context(tc.tile_pool(name="rowap", bufs=2))
        sqp = ctx.enter_context(tc.tile_pool(name="sqp", bufs=1))
        gxp = ctx.enter_context(tc.tile_pool(name="gxp", bufs=2))
        w2sp = ctx.enter_context(tc.tile_pool(name="w2sp", bufs=1))
        bcsb = ctx.enter_context(tc.tile_pool(name="bcsb", bufs=2))
        obp = ctx.enter_context(tc.tile_pool(name="obp", bufs=1))
        # PSUM: 4 matmul banks + 2 stats banks + 2 broadcast banks = 8
        mmps = ctx.enter_context(tc.tile_pool(name="mmps", bufs=4, space="PSUM"))
        smps = ctx.enter_context(tc.tile_pool(name="smps", bufs=2, space="PSUM"))
        cnvps = ctx.enter_context(tc.tile_pool(name="cnvps", bufs=2,
                                               space="PSUM"))
        bcps = mmps

        # ---- constants (small ones first so conv starts immediately) ----
        dww = singles.tile([128, CC, K], F32)
        nc.sync.dma_start(dww[:], dww_d)
        dwb = singles.tile([128, CC], F32)
        nc.sync.dma_start(dwb[:], dwb_d)
        dwdg = singles.tile([128, CC, K, 128], F32R)
        for _ci in range(CC):
            nc.sync.dma_start(dwdg[:, _ci], dwdg_d[:, _ci])

        xv = x_d.rearrange("b (cc p) t -> b p cc t", p=128)
        ov = out_d.rearrange("b (cc p) t -> b p cc t", p=128)

        # state carried between emission steps
        xh_t = [None] * G          # conv x tiles (with halo)
        xh_meta = [None] * G       # (lo, off, xn)
        y_t = [None] * G
        ysq_t = [None] * G
        ysc_t = [None] * G
        h_t = [None] * G
        row_t = [None] * G         # [M_row, A_row]
        sm_t = [None] * G          # [sumy, sumsq]
        bc_t = [None] * G          # [bbc_ps, abc_ps]
        xr_t = [None] * G
        gxpart = [None] * BPC
        ss_t = [None] * BPC
        w2x = [None] * BPC         # scaled w2 for mm2 (lhsT)

        def emit_x_dma(g):
            bb, tt = divmod(g, TC)
            t0 = tt * TN
            lo = max(0, t0 - 3)
            hi = min(T, t0 + TN + 3)
            xn = hi - lo
            j0 = 3 - (t0 - lo)           # 3 at the left edge, else 0
            xhs = []
            for ci in range(CC):
                xh = xhp.tile([128, XH], F32R, tag="xh", name="xh")
                if j0 > 0:
                    nc.vector.memset(xh[:, 0:j0].bitcast(U32), 0)
                if j0 + xn < XH:
                    nc.vector.memset(xh[:, j0 + xn:XH].bitcast(U32), 0)
                nc.sync.dma_start(xh[:, j0:j0 + xn], xv[bb, :, ci, lo:hi])
                xhs.append(xh)
            xh_t[g] = xhs

        def emit_conv(g, cis):
            # conv on PE: 7 diag-matmul taps accumulate in PSUM, Act drains
            # with the depthwise bias; ysq on DVE for the sumsq stats.
            bb, tt = divmod(g, TC)
            if y_t[g] is None:
                y_t[g] = yp.tile([128, CC, TN], F32R, tag="y", name="y")
                ysq_t[g] = ysqp.tile([128, CC, TN], BF16, tag="ysq", name="ysq")
            y = y_t[g]
            ysq = ysq_t[g]
            for ci in cis:
                xh = xh_t[g][ci]
                ps = cnvps.tile([128, TN], F32, tag="cv", name="cv")
                for k in range(K):
                    nc.tensor.matmul(ps[:], dwdg[:, ci, k, :],
                                     xh[:, k:k + TN],
                                     start=(k == 0), stop=(k == K - 1))
                nc.scalar.activation(y[:, ci, :], ps[:], AF.Identity,
                                     bias=dwb[:, ci:ci + 1])
                nc.vector.tensor_mul(ysq[:, ci, :], y[:, ci, :].bitcast(F32),
                                     y[:, ci, :].bitcast(F32))

        def emit_stats_a(g):
            y = y_t[g]
            ysq = ysq_t[g]
            sumy = smps.tile([1, TN], F32, tag="sm", name="sumy")
            sumsq = smps.tile([1, TN], F32, tag="sm", name="sumsq")
            for ci in (0, 1):
                nc.tensor.matmul(sumy[:], ones_col[:], y[:, ci, :],
                                 start=(ci == 0), stop=False)
                nc.tensor.matmul(sumsq[:], ones_col_bf[:], ysq[:, ci, :],
                                 start=(ci == 0), stop=False)
            sm_t[g] = [sumy, sumsq]

        def emit_sumy(g):
            y = y_t[g]
            sumy = sm_t[g][0]
            for ci in (2, 3):
                nc.tensor.matmul(sumy[:], ones_col[:], y[:, ci, :],
                                 start=False, stop=(ci == CC - 1))
            # M_row = -mu; msq/meps gated only on sumy -> run early on Pool
            M_row = rowmp.tile([1, TN], F32R, tag="rowm", name="mrow")
            nc.vector.tensor_scalar(M_row[:], sumy[:], -1.0 / C, None, OP.mult)
            msq = rowp.tile([1, TN], F32, tag="row", name="msq")
            nc.scalar.activation(msq[:], M_row[:].bitcast(F32), AF.Square)
            row_t[g] = [M_row, None, msq]

        def emit_sumsq(g):
            ysq = ysq_t[g]
            sumsq = sm_t[g][1]
            for ci in (2, 3):
                nc.tensor.matmul(sumsq[:], ones_col_bf[:], ysq[:, ci, :],
                                 start=False, stop=(ci == CC - 1))
            meps = row_t[g][2]
            vpe = rowp.tile([1, TN], F32, tag="row", name="vpe")
            nc.vector.scalar_tensor_tensor(vpe[:], sumsq[:], 1.0 / C, meps[:],
                                           OP.mult, OP.subtract)
            # A = rsqrt(vpe): bit-trick seed + 1 Newton step (no act table)
            sh = rowap.tile([1, TN], U32, tag="rowa", name="sh")
            nc.vector.tensor_scalar(sh[:], vpe[:].bitcast(U32), one_row[:],
                                    None, OP.logical_shift_right)
            a0 = rowap.tile([1, TN], U32, tag="rowa", name="a0")
            nc.vector.scalar_tensor_tensor(a0[:], magic_row[:], 1.0, sh[:],
                                           OP.bypass, OP.subtract)
            t1 = rowp.tile([1, TN], F32, tag="row", name="t1")
            nc.scalar.activation(t1[:], a0[:].bitcast(F32), AF.Square)
            t2 = rowp.tile([1, TN], F32, tag="row", name="t2")
            nc.vector.tensor_mul(t2[:], t1[:], vpe[:])
            t3 = rowp.tile([1, TN], F32, tag="row", name="t3")
            nc.vector.tensor_scalar(t3[:], t2[:], -0.5, 1.5, OP.mult, OP.add)
            A_row = rowp.tile([1, TN], F32R, tag="row", name="arow")
            nc.vector.tensor_mul(A_row[:], a0[:].bitcast(F32), t3[:])
            row_t[g][1] = A_row

        def emit_bbc(g):
            M_row = row_t[g][0]
            bbc_ps = bcps.tile([128, TN], F32, tag="mm", name="bbcps")
            nc.tensor.matmul(bbc_ps[:], ones_row[:], M_row[:],
                             start=True, stop=True)
            bbc = bcsb.tile([128, TN], F32, tag="bcs", name="bbc")
            nc.scalar.activation(bbc[:], bbc_ps[:], AF.Copy)
            bc_t[g] = [bbc, None]
            # y += bbc (in place on DVE)
            y = y_t[g]
            for ci in range(CC):
                nc.vector.tensor_add(y[:, ci, :], y[:, ci, :].bitcast(F32),
                                     bbc[:])

        def emit_abc(g):
            A_row = row_t[g][1]
            abc_ps = bcps.tile([128, TN], F32, tag="mm", name="abcps")
            nc.tensor.matmul(abc_ps[:], ones_row[:], A_row[:],
                             start=True, stop=True)
            abc = bcsb.tile([128, TN], F32, tag="bcs", name="abc")
            nc.vector.tensor_copy(abc[:], abc_ps[:])
            bc_t[g][1] = abc
            # ysc = y * abc  (pool reads PSUM)
            y = y_t[g]
            ysc = yscp.tile([128, CC, TN], F32R, tag="ysc", name="ysc")
            for ci in range(CC):
                nc.vector.tensor_mul(ysc[:, ci, :], y[:, ci, :].bitcast(F32),
                                     abc[:])
            ysc_t[g] = ysc

        def emit_mm1(g, interject=None):
            bb, tt = divmod(g, TC)
            ysc = ysc_t[g]
            h = hp.tile([128, IC, TN], F16, tag="h", name="h")
            h_t[g] = h
            for ii in range(IC):
                if interject is not None:
                    interject(ii)
                ph = mmps.tile([128, TN], F32, tag="mm", name="ph")
                isl = slice(ii * 128, (ii + 1) * 128)
                for ci in range(CC):
                    nc.tensor.matmul(ph[:], w1pT[:, ci, isl], ysc[:, ci, :],
                                     start=(ci == 0), stop=(ci == CC - 1))
                nc.scalar.activation(h[:, ii, :], ph[:], AF.Gelu,
                                     bias=b1p[:, ii:ii + 1])
                # GRN partial: sum_t h^2 for this (ii, t); DVE/Act split
                ndve = 5
                if ii < ndve:
                    sqd = sqp.tile([128, TN], F16, tag="sq", name="sqd")
                    nc.vector.scalar_tensor_tensor(
                        sqd[:], h[:, ii, :], 1.0, h[:, ii, :],
                        OP.bypass, OP.mult,
                        accum_out=gxpart[bb][:, ii, tt:tt + 1])
                else:
                    sqd = sqp.tile([128, TN], F16, tag="sqa", name="sqa")
                    nc.scalar.activation(
                        sqd[:], h[:, ii, :], AF.Square,
                        accum_out=gxpart[bb][:, ii, tt:tt + 1])

        def emit_grn(bb):
            gxsq = gxp.tile([128, IC], F32, tag="gx2", name="gxsq")
            nc.vector.tensor_reduce(gxsq[:], gxpart[bb][:],
                                    axis=mybir.AxisListType.X, op=OP.add)
            gsh = gxp.tile([128, IC], U32, tag="gxa", name="gsh")
            nc.vector.tensor_scalar(gsh[:], gxsq[:].bitcast(U32), one_col[:],
                                    None, OP.logical_shift_right)
            ga0 = gxp.tile([128, IC], U32, tag="gxa", name="ga0")
            nc.vector.scalar_tensor_tensor(ga0[:], magic_col[:], 1.0, gsh[:],
                                           OP.bypass, OP.subtract)
            gt1 = gxp.tile([128, IC], F32, tag="gxn", name="gt1")
            nc.vector.tensor_mul(gt1[:], ga0[:].bitcast(F32),
                                 ga0[:].bitcast(F32))
            gt2 = gxp.tile([128, IC], F32, tag="gxn", name="gt2")
            nc.vector.tensor_mul(gt2[:], gt1[:], gxsq[:])
            gt3 = gxp.tile([128, IC], F32, tag="gxn", name="gt3")
            nc.vector.tensor_scalar(gt3[:], gt2[:], -0.5, 1.5, OP.mult, OP.add)
            grs = gxp.tile([128, IC], F32, tag="gxn", name="grs")
            nc.vector.tensor_mul(grs[:], ga0[:].bitcast(F32), gt3[:])
            gx = gxp.tile([128, IC], F32R, tag="gx2", name="gx")
            nc.vector.tensor_mul(gx[:], gxsq[:], grs[:])
            gsum = smps.tile([1, IC], F32, tag="sm", name="gsum")
            nc.tensor.matmul(gsum[:], ones_col[:], gx[:], start=True, stop=True)
            gtot = gxp.tile([1, 1], F32, tag="gx3", name="gtot")
            nc.vector.tensor_reduce(gtot[:], gsum[:],
                                    axis=mybir.AxisListType.X, op=OP.add)
            dinv = gxp.tile([1, 1], F32, tag="gx3", name="dinv")
            nc.vector.tensor_scalar(dinv[:], gtot[:], 1.0 / I, GRN_EPS,
                                    OP.mult, OP.add)
            d_row = gxp.tile([1, 1], F32R, tag="gx3", name="drow")
            nc.vector.reciprocal(d_row[:], dinv[:])
            dbc = gxp.tile([128, 1], F32, tag="gx4", name="dbc")
            nc.gpsimd.partition_broadcast(dbc[:], d_row[:].bitcast(F32))
            ss = gxp.tile([128, IC], F32, tag="gx4", name="ss")
            nc.vector.scalar_tensor_tensor(ss[:], gx[:].bitcast(F32), dbc[:],
                                           grng[:], OP.mult, OP.mult)
            nc.vector.tensor_scalar(ss[:], ss[:], 1.0, None, OP.add)
            ss_t[bb] = ss

        def emit_w2scale(bb, iis):
            # w2 * ss, alternating Pool/DVE so mm2 is fed quickly;
            # b0 into a fresh buffer, b1 in place
            ss = ss_t[bb]
            if bb == 0 and w2x[0] is None:
                w2x[0] = w2sp.tile([128, IC, C], F16, tag="w2s", name="w2s")
            dst = w2x[0] if bb == 0 else w2t
            for ii in iis:
                if ii % 2 == 0:
                    nc.vector.tensor_scalar(dst[:, ii, :], w2t[:, ii, :],
                                            ss[:, ii:ii + 1], None, OP.mult)
                else:
                    nc.scalar.activation(dst[:, ii, :], w2t[:, ii, :],
                                         AF.Copy, scale=ss[:, ii:ii + 1])
            if bb == 1:
                w2x[1] = w2t

        def emit_xres_dma(g):
            bb, tt = divmod(g, TC)
            xrs = []
            for ci in range(CC):
                xr = xrp.tile([128, TN], F32, tag="xr", name="xr")
                nc.sync.dma_start(
                    xr[:], xv[bb, :, ci, tt * TN:(tt + 1) * TN].bitcast(F32))
                xrs.append(xr)
            xr_t[g] = xrs

        def emit_mm2(g):
            bb, tt = divmod(g, TC)
            h = h_t[g]
            w2 = w2x[bb]
            for ci in range(CC):
                po = mmps.tile([128, TN], F32, tag="mm", name="po")
                csl = slice(ci * 128, (ci + 1) * 128)
                for ii in range(IC):
                    nc.tensor.matmul(po[:], w2[:, ii, csl], h[:, ii, :],
                                     start=(ii == 0), stop=(ii == IC - 1))
                # drain per ci: (po + b2p) + x_res fused on DVE
                o_sb = obp.tile([128, TN], F32, tag="ob", name="osb")
                nc.vector.scalar_tensor_tensor(
                    o_sb[:], po[:], b2p[:, ci:ci + 1], xr_t[g][ci][:],
                    OP.add, OP.add)
                nc.sync.dma_start(ov[bb][:, ci, tt * TN:(tt + 1) * TN],
                                  o_sb[:])

        # ---------------- emission schedule ----------------
        emit_x_dma(0)
        emit_x_dma(1)

        # weights after the first x tiles so conv starts promptly
        w1pT = singles.tile([128, CC, I], F32R)
        nc.sync.dma_start(w1pT[:], w1pT_d.rearrange("(cc p) i -> p cc i", p=128))
        b1p = singles.tile([128, IC], F32)
        nc.sync.dma_start(b1p[:], b1p_d)
        b2p = singles.tile([128, CC], F32)
        nc.sync.dma_start(b2p[:], b2p_d)
        grng = singles.tile([128, IC], F32)
        nc.sync.dma_start(grng[:], grng_d)
        w2t = singles.tile([128, IC, C], F16)
        nc.scalar.dma_start(w2t[:], w2T_d.rearrange("(ic p) c -> p ic c", p=128))

        onesf = singles.tile([128, 1], F32)
        nc.vector.memset(onesf[:], 1.0)
        ones_col = singles.tile([128, 1], F32R)   # stats lhsT (K=128, M=1)
        nc.vector.tensor_copy(ones_col[:], onesf[:])
        onesrf = singles.tile([1, 128], F32)
        nc.vector.memset(onesrf[:], 1.0)
        ones_row = singles.tile([1, 128], F32R)   # bcast lhsT (K=1, M=128)
        nc.vector.tensor_copy(ones_row[:], onesrf[:])
        ones_col_bf = singles.tile([128, 1], BF16)
        nc.vector.tensor_copy(ones_col_bf[:], onesf[:])
        magic_row = singles.tile([1, TN], U32)
        nc.vector.memset(magic_row[:], 0x5F3759DF)
        one_row = singles.tile([1, 1], U32)
        nc.vector.memset(one_row[:], 1)
        magic_col = singles.tile([128, IC], U32)
        nc.vector.memset(magic_col[:], 0x5F3759DF)
        one_col = singles.tile([128, 1], U32)
        nc.vector.memset(one_col[:], 1)

        gxpart[0] = gxp.tile([128, IC, TC], F32, tag="gxpart", name="gxp0")
        gxpart[1] = gxp.tile([128, IC, TC], F32, tag="gxpart", name="gxp1")

        # tile 0 prologue (PE idles here once)
        for _ci in range(CC):
            emit_conv(0, (_ci,))
        emit_stats_a(0)
        emit_sumy(0)
        emit_sumsq(0)
        emit_bbc(0)
        emit_abc(0)
        emit_conv(1, (0, 1))   # half of tile 1's conv ahead of block 0
        emit_stats_a(1)

        for g in range(G):
            # prefetch x two tiles ahead; second half of next tile's conv
            if g + 2 < G:
                emit_x_dma(g + 2)
            if g + 1 < G:
                emit_conv(g + 1, (2, 3))
            if g >= 3 and g - 3 < TC:          # x_res for b0's mm2 tiles
                emit_xres_dma(g - 3)

            gn = g + 1 if g + 1 < G else None

            def interject(ii, gn=gn, g=g):
                if gn is not None:
                    if ii == 2:
                        emit_sumy(gn)
                    elif ii == 4:
                        emit_sumsq(gn)
                    elif ii == 6:
                        emit_bbc(gn)
                    elif ii == 8:
                        emit_abc(gn)
                if g == 4 and ii in (3, 7, 11):    # w2s for b0, in chunks
                    emit_w2scale(0, range(ii - 3, ii + 1))

            emit_mm1(g, interject)

            if g == TC - 1:
                emit_grn(0)
            if TC <= g < 2 * TC:
                emit_mm2(g - TC)
            if g + 2 < G:
                emit_conv(g + 2, (0, 1))   # first half of conv, a block early
                emit_stats_a(g + 2)

        emit_grn(1)
        emit_w2scale(1, range(IC))
        for t in range(TC):
            emit_xres_dma(TC + t)
            emit_mm2(TC + t)


def _diag_taps(dw):
    # [128, CC, K, 128]: lhsT diag matrices so conv taps run on PE
    out = np.zeros((128, CC, K, 128), np.float32)
    for ci in range(CC):
        for p in range(128):
            out[p, ci, :, p] = dw[ci * 128 + p]
    return out


def _host_prep(inputs):
    w1 = inputs["w1"].astype(np.float64)
    ln_g = inputs["ln_g"].astype(np.float64)
    ln_b = inputs["ln_b"].astype(np.float64)
    w2 = inputs["w2"].astype(np.float64)
    w1p = w1 * ln_g[None, :]                         # [I, C]
    prep = {
        "w1pT": np.ascontiguousarray(w1p.T).astype(np.float32),
        "b1p": (inputs["b1"].astype(np.float64) + w1 @ ln_b)
               .astype(np.float32).reshape(IC, 128).T.copy(),
        "w2T": np.ascontiguousarray(w2.T).astype(np.float16),
        "b2p": (inputs["b2"].astype(np.float64)
                + w2 @ inputs["grn_b"].astype(np.float64))
               .astype(np.float32).reshape(CC, 128).T.copy(),
        "grng": inputs["grn_g"].reshape(IC, 128).T.copy().astype(np.float32),
        "dww": inputs["dw_w"].reshape(C, K).reshape(CC, 128, K)
               .transpose(1, 0, 2).copy().astype(np.float32),
        "dwdg": _diag_taps(inputs["dw_w"].reshape(C, K)),
        "dwb": inputs["dw_b"].reshape(CC, 128).T.copy().astype(np.float32),
    }
    return prep


def run(inputs, trace=False, **kw):
    if "nc" not in _CACHE:
        _CACHE["nc"] = _build()
    nc = _CACHE["nc"]
    prep = _host_prep(inputs)
    x = np.asarray(inputs["x"], dtype=np.float32)
    in_maps = []
    for c in range(NCORES):
        m = dict(prep)
        m["x"] = np.ascontiguousarray(x[c * BPC:(c + 1) * BPC])
        in_maps.append(m)
    res = run_bass_kernel_spmd(nc, in_maps, core_ids=list(range(NCORES)),
                               trace=trace, **kw)
    out = np.concatenate([r["out"] for r in res.results], axis=0)
    return out, res


def kernel(**inputs):
    out, _ = run(inputs)
    return out


# revision 41
# speedup vs baseline: 9.6466x; 2.8739x over previous
"""ConvNeXt block (dwconv7 -> LN -> pwconv1 -> GELU -> GRN -> pwconv2 -> residual)
for Trainium2, batch-parallel across 8 NeuronCores (2 batches per core).

Self-contained: hardcodes shapes B=16, C=512, T=2048, I=1536, K=7.

v2 design (vs v1 baseline):
  - h kept entirely in SBUF (f16 tiles, 5-deep pool) -- no DRAM spill.
  - LN mean handled by a second broadcast matmul (bbc = -mu) + in-place
    pool add, replacing the per-ii rank-1 W1s matmuls.
  - 8 global t-tiles (2 batches x 4); per-tile work balanced across
    DVE/Pool/Act so each engine carries ~12us per 12.4us PE block:
      DVE:  conv taps {1,3,5}, GRN squares ii 0..5, row chain
      Pool: conv taps {0,2,4,6}, ysq, LN apply, w2 scaling, residual add
      Act:  GELU, GRN squares ii 6..11, stdv
  - mm2 of batch 0 interleaves with mm1 of batch 1; GRN chains hide under
    matmuls; conv is split across block boundaries for early LN stats.
  - All DMA issued from SP/Act HWDGE queues (none from Pool/SWDGE).
  - w2 scaled into a separate buffer for b0, in place for b1 (single load).

Math folding (host-side, weight-sized only):
  LN:  y_ln = (y - mu_t) * A_t * ln_g + ln_b      (A_t = rsqrt(var_t + eps))
  mm1: h = GELU( w1p @ ((y - mu)*A) + b1p ),  w1p = w1*ln_g, b1p = b1 + w1@ln_b
  GRN: h' = h * ss[i] + grn_b[i],  ss = 1 + grn_g * gx * d,
       gx = sqrt(sum_t h^2), d = 1/(mean_i gx + eps)
  mm2: out = (w2 * ss).T-contracted with h + (b2 + w2 @ grn_b) + residual
"""
import sys

sys.path.insert(0, "/opt/trn_rl_repo")

import numpy as np
import concourse.bacc as bacc
import concourse.tile as tile
from concourse import mybir
from concourse.bass_utils import run_bass_kernel_spmd

F32 = mybir.dt.float32
F32R = mybir.dt.float32r
F16 = mybir.dt.float16
BF16 = mybir.dt.bfloat16
U32 = mybir.dt.uint32
AF = mybir.ActivationFunctionType
OP = mybir.AluOpType

B, C, T, I, K = 16, 512, 2048, 1536, 7
NCORES = 8
BPC = B // NCORES          # batches per core
CC = C // 128              # 4 c-chunks
IC = I // 128              # 12 i-chunks
TC = T // 512              # 4 t-chunks per batch
TN = 512                   # matmul free-dim tile
G = BPC * TC               # 8 global tiles
LN_EPS = 1e-6
GRN_EPS = 1e-6

_CACHE = {}


def _build(trace_sim=False, reps=1):
    nc = bacc.Bacc("TRN2", target_bir_lowering=False, debug=False,
                   num_devices=NCORES)
    dram = {}

    def din(name, shape, dt=F32):
        dram[name] = nc.dram_tensor(name, shape, dt, kind="ExternalInput").ap()
        return dram[name]

    x_d = din("x", [BPC, C, T], F32R)                # per-core batches
    w1pT_d = din("w1pT", [C, I], F32R)               # (w1*ln_g).T  [c, i]
    b1p_d = din("b1p", [128, IC])                    # col-chunked b1p
    w2T_d = din("w2T", [I, C], F16)                  # w2.T  [i, c]
    b2p_d = din("b2p", [128, CC])                    # col-chunked b2 + w2@grn_b
    grng_d = din("grng", [128, IC])                  # col-chunked grn_g
    dww_d = din("dww", [128, CC, K])                 # depthwise taps per c-chunk
    dwdg_d = din("dwdg", [128, CC, K, 128], F32R)    # diag taps for PE conv
    dwb_d = din("dwb", [128, CC])                    # depthwise bias per c-chunk
    out_d = nc.dram_tensor("out", [BPC, C, T], F32, kind="ExternalOutput").ap()

    with tile.TileContext(nc, trace_sim=trace_sim) as tc:
        for _ in range(reps):
            _kernel_body(nc, tc, x_d, w1pT_d, b1p_d, w2T_d, b2p_d,
                         grng_d, dww_d, dwb_d, dwdg_d, out_d)
    nc.compile()
    return nc


def _kernel_body(nc, tc, x_d, w1pT_d, b1p_d, w2T_d, b2p_d,
                 grng_d, dww_d, dwb_d, dwdg_d, out_d):
    from contextlib import ExitStack
    XH = TN + 6
    ctx = ExitStack()
    with ctx:
        ctx.enter_context(nc.allow_low_precision(
            reason="f32r matmul operand rounding is intentional"))
        singles = ctx.enter_context(tc.tile_pool(name="singles", bufs=1))
        xhp = ctx.enter_context(tc.tile_pool(name="xhp", bufs=7))
        xrp = ctx.enter_context(tc.tile_pool(name="xrp", bufs=3))
        yp = ctx.enter_context(tc.tile_pool(name="yp", bufs=2))
        ysqp = ctx.enter_context(tc.tile_pool(name="ysqp", bufs=1))
        yscp = ctx.enter_context(tc.tile_pool(name="yscp", bufs=2))
        hp = ctx.enter_context(tc.tile_pool(name="hp", bufs=5))
        rowp = ctx.enter_context(tc.tile_pool(name="rowp", bufs=3))
        rowmp = ctx.enter_context(tc.tile_pool(name="rowmp", bufs=2))
        rowap = ctx.enter_context(tc.tile_pool(name="rowap", bufs=2))
        sqp = ctx.enter_context(tc.tile_pool(name="sqp", bufs=1))
        gxp = ctx.enter_context(tc.tile_pool(name="gxp", bufs=2))
        w2sp = ctx.enter_context(tc.tile_pool(name="w2sp", bufs=1))
        bcsb = ctx.enter_context(tc.tile_pool(name="bcsb", bufs=2))
        obp = ctx.enter_context(tc.tile_pool(name="obp", bufs=2))
        # PSUM: 4 matmul banks + 2 stats banks + 2 broadcast banks = 8
        mmps = ctx.enter_context(tc.tile_pool(name="mmps", bufs=4, space="PSUM"))
        smps = ctx.enter_context(tc.tile_pool(name="smps", bufs=2, space="PSUM"))
        cnvps = ctx.enter_context(tc.tile_pool(name="cnvps", bufs=2,
                                               space="PSUM"))
        bcps = mmps

        # ---- constants (small ones first so conv starts immediately) ----
        dww = singles.tile([128, CC, K], F32)
        nc.sync.dma_start(dww[:], dww_d)
        dwb = singles.tile([128, CC], F32)
        nc.sync.dma_start(dwb[:], dwb_d)
        dwdg = singles.tile([128, CC, K, 128], F32R)

        xv = x_d.rearrange("b (cc p) t -> b p cc t", p=128)
        ov = out_d.rearrange("b (cc p) t -> b p cc t", p=128)

        # state carried between emission steps
        xh_t = [None] * G          # conv x tiles (with halo)
        xh_meta = [None] * G       # (lo, off, xn)
        y_t = [None] * G
        ysq_t = [None] * G
        ysc_t = [None] * G
        h_t = [None] * G
        row_t = [None] * G         # [M_row, A_row]
        sm_t = [None] * G          # [sumy, sumsq]
        bc_t = [None] * G          # [bbc_ps, abc_ps]
        xr_t = [None] * G
        gxpart = [None] * BPC
        ss_t = [None] * BPC
        w2x = [None] * BPC         # scaled w2 for mm2 (lhsT)

        def emit_x_dma(g):
            bb, tt = divmod(g, TC)
            t0 = tt * TN
            lo = max(0, t0 - 3)
            hi = min(T, t0 + TN + 3)
            xn = hi - lo
            j0 = 3 - (t0 - lo)           # 3 at the left edge, else 0
            xhs = []
            for ci in range(CC):
                xh = xhp.tile([128, XH], F32R, tag="xh", name="xh")
                if j0 > 0:
                    nc.vector.memset(xh[:, 0:j0].bitcast(U32), 0)
                if j0 + xn < XH:
                    nc.vector.memset(xh[:, j0 + xn:XH].bitcast(U32), 0)
                nc.sync.dma_start(xh[:, j0:j0 + xn], xv[bb, :, ci, lo:hi])
                xhs.append(xh)
            xh_t[g] = xhs

        def emit_conv(g, cis):
            # conv on PE: 7 diag-matmul taps accumulate in PSUM, Act drains
            # with the depthwise bias; ysq on DVE for the sumsq stats.
            bb, tt = divmod(g, TC)
            if y_t[g] is None:
                y_t[g] = yp.tile([128, CC, TN], F32R, tag="y", name="y")
                ysq_t[g] = ysqp.tile([128, CC, TN], BF16, tag="ysq", name="ysq")
            y = y_t[g]
            ysq = ysq_t[g]
            for ci in cis:
                xh = xh_t[g][ci]
                ps = cnvps.tile([128, TN], F32, tag="cv", name="cv")
                for k in range(1, K):
                    nc.tensor.matmul(ps[:], dwdg[:, ci, k, :],
                                     xh[:, k:k + TN],
                                     start=(k == 1), stop=(k == K - 1))
                nc.scalar.activation(y[:, ci, :], ps[:], AF.Identity,
                                     bias=dwb[:, ci:ci + 1])
                # tap k=0 on DVE (PE is the critical engine)
                nc.vector.scalar_tensor_tensor(
                    y[:, ci, :], xh[:, 0:TN].bitcast(F32),
                    dww[:, ci, 0:1], y[:, ci, :].bitcast(F32),
                    OP.mult, OP.add)
                nc.vector.tensor_mul(ysq[:, ci, :], y[:, ci, :].bitcast(F32),
                                     y[:, ci, :].bitcast(F32))

        def emit_stats_a(g):
            y = y_t[g]
            ysq = ysq_t[g]
            sumy = smps.tile([1, TN], F32, tag="sm", name="sumy")
            sumsq = smps.tile([1, TN], F32, tag="sm", name="sumsq")
            for ci in (0, 1):
                nc.tensor.matmul(sumy[:], ones_col[:], y[:, ci, :],
                                 start=(ci == 0), stop=False)
                nc.tensor.matmul(sumsq[:], ones_col_bf[:], ysq[:, ci, :],
                                 start=(ci == 0), stop=False)
            sm_t[g] = [sumy, sumsq]

        def emit_sumy(g):
            y = y_t[g]
            sumy = sm_t[g][0]
            for ci in (2, 3):
                nc.tensor.matmul(sumy[:], ones_col[:], y[:, ci, :],
                                 start=False, stop=(ci == CC - 1))
            # M_row = -mu; msq/meps gated only on sumy -> run early on Pool
            M_row = rowmp.tile([1, TN], F32R, tag="rowm", name="mrow")
            nc.vector.tensor_scalar(M_row[:], sumy[:], -1.0 / C, None, OP.mult)
            msq = rowp.tile([1, TN], F32, tag="row", name="msq")
            nc.scalar.activation(msq[:], M_row[:].bitcast(F32), AF.Square)
            row_t[g] = [M_row, None, msq]

        def emit_sumsq(g):
            ysq = ysq_t[g]
            sumsq = sm_t[g][1]
            for ci in (2, 3):
                nc.tensor.matmul(sumsq[:], ones_col_bf[:], ysq[:, ci, :],
                                 start=False, stop=(ci == CC - 1))
            meps = row_t[g][2]
            vpe = rowp.tile([1, TN], F32, tag="row", name="vpe")
            nc.vector.scalar_tensor_tensor(vpe[:], sumsq[:], 1.0 / C, meps[:],
                                           OP.mult, OP.subtract)
            # A = rsqrt(vpe): bit-trick seed + 1 Newton step (no act table)
            sh = rowap.tile([1, TN], U32, tag="rowa", name="sh")
            nc.vector.tensor_scalar(sh[:], vpe[:].bitcast(U32), one_row[:],
                                    None, OP.logical_shift_right)
            a0 = rowap.tile([1, TN], U32, tag="rowa", name="a0")
            nc.vector.scalar_tensor_tensor(a0[:], magic_row[:], 1.0, sh[:],
                                           OP.bypass, OP.subtract)
            t1 = rowp.tile([1, TN], F32, tag="row", name="t1")
            nc.scalar.activation(t1[:], a0[:].bitcast(F32), AF.Square)
            t2 = rowp.tile([1, TN], F32, tag="row", name="t2")
            nc.vector.tensor_mul(t2[:], t1[:], vpe[:])
            t3 = rowp.tile([1, TN], F32, tag="row", name="t3")
            nc.vector.tensor_scalar(t3[:], t2[:], -0.5, 1.5, OP.mult, OP.add)
            A_row = rowp.tile([1, TN], F32R, tag="row", name="arow")
            nc.vector.tensor_mul(A_row[:], a0[:].bitcast(F32), t3[:])
            row_t[g][1] = A_row

        def emit_bbc(g):
            M_row = row_t[g][0]
            bbc_ps = bcps.tile([128, TN], F32, tag="mm", name="bbcps")
            nc.tensor.matmul(bbc_ps[:], ones_row[:], M_row[:],
                             start=True, stop=True)
            bbc = bcsb.tile([128, TN], F32, tag="bcs", name="bbc")
            nc.scalar.activation(bbc[:], bbc_ps[:], AF.Copy)
            bc_t[g] = [bbc, None]
            # y += bbc (in place on DVE)
            y = y_t[g]
            for ci in range(CC):
                nc.vector.tensor_add(y[:, ci, :], y[:, ci, :].bitcast(F32),
                                     bbc[:])

        def emit_abc(g):
            A_row = row_t[g][1]
            abc_ps = bcps.tile([128, TN], F32, tag="mm", name="abcps")
            nc.tensor.matmul(abc_ps[:], ones_row[:], A_row[:],
                             start=True, stop=True)
            abc = bcsb.tile([128, TN], F32, tag="bcs", name="abc")
            nc.vector.tensor_copy(abc[:], abc_ps[:])
            bc_t[g][1] = abc
            # ysc = y * abc  (pool reads PSUM)
            y = y_t[g]
            ysc = yscp.tile([128, CC, TN], F32R, tag="ysc", name="ysc")
            for ci in range(CC):
                nc.vector.tensor_mul(ysc[:, ci, :], y[:, ci, :].bitcast(F32),
                                     abc[:])
            ysc_t[g] = ysc

        def emit_mm1(g, interject=None):
            bb, tt = divmod(g, TC)
            ysc = ysc_t[g]
            h = hp.tile([128, IC, TN], F16, tag="h", name="h")
            h_t[g] = h
            for ii in range(IC):
                if interject is not None:
                    interject(ii)
                ph = mmps.tile([128, TN], F32, tag="mm", name="ph")
                isl = slice(ii * 128, (ii + 1) * 128)
                for ci in range(CC):
                    nc.tensor.matmul(ph[:], w1pT[:, ci, isl], ysc[:, ci, :],
                                     start=(ci == 0), stop=(ci == CC - 1))
                nc.scalar.activation(h[:, ii, :], ph[:], AF.Gelu,
                                     bias=b1p[:, ii:ii + 1])
                # GRN partial: sum_t h^2 for this (ii, t); DVE/Act split
                ndve = 5
                if ii < ndve:
                    sqd = sqp.tile([128, TN], F16, tag="sq", name="sqd")
                    nc.vector.scalar_tensor_tensor(
                        sqd[:], h[:, ii, :], 1.0, h[:, ii, :],
                        OP.bypass, OP.mult,
                        accum_out=gxpart[bb][:, ii, tt:tt + 1])
                else:
                    sqd = sqp.tile([128, TN], F16, tag="sqa", name="sqa")
                    nc.scalar.activation(
                        sqd[:], h[:, ii, :], AF.Square,
                        accum_out=gxpart[bb][:, ii, tt:tt + 1])

        def emit_grn(bb):
            gxsq = gxp.tile([128, IC], F32, tag="gx2", name="gxsq")
            nc.vector.tensor_reduce(gxsq[:], gxpart[bb][:],
                                    axis=mybir.AxisListType.X, op=OP.add)
            gsh = gxp.tile([128, IC], U32, tag="gxa", name="gsh")
            nc.vector.tensor_scalar(gsh[:], gxsq[:].bitcast(U32), one_col[:],
                                    None, OP.logical_shift_right)
            ga0 = gxp.tile([128, IC], U32, tag="gxa", name="ga0")
            nc.vector.scalar_tensor_tensor(ga0[:], magic_col[:], 1.0, gsh[:],
                                           OP.bypass, OP.subtract)
            gt1 = gxp.tile([128, IC], F32, tag="gxn", name="gt1")
            nc.vector.tensor_mul(gt1[:], ga0[:].bitcast(F32),
                                 ga0[:].bitcast(F32))
            gt2 = gxp.tile([128, IC], F32, tag="gxn", name="gt2")
            nc.vector.tensor_mul(gt2[:], gt1[:], gxsq[:])
            gt3 = gxp.tile([128, IC], F32, tag="gxn", name="gt3")
            nc.vector.tensor_scalar(gt3[:], gt2[:], -0.5, 1.5, OP.mult, OP.add)
            grs = gxp.tile([128, IC], F32, tag="gxn", name="grs")
            nc.vector.tensor_mul(grs[:], ga0[:].bitcast(F32), gt3[:])
            gx = gxp.tile([128, IC], F32R, tag="gx2", name="gx")
            nc.vector.tensor_mul(gx[:], gxsq[:], grs[:])
            gsum = smps.tile([1, IC], F32, tag="sm", name="gsum")
            nc.tensor.matmul(gsum[:], ones_col[:], gx[:], start=True, stop=True)
            gtot = gxp.tile([1, 1], F32, tag="gx3", name="gtot")
            nc.vector.tensor_reduce(gtot[:], gsum[:],
                                    axis=mybir.AxisListType.X, op=OP.add)
            dinv = gxp.tile([1, 2], F32, tag="gx3", name="dinv")
            nc.vector.tensor_scalar(dinv[:, 0:1], gtot[:], 1.0 / I, GRN_EPS,
                                    OP.mult, OP.add)
            nc.vector.tensor_scalar(dinv[:, 1:2], gtot[:], 1.0 / I, GRN_EPS,
                                    OP.mult, OP.add)
            d_row = gxp.tile([1, 2], F32R, tag="gx3", name="drow")
            nc.vector.reciprocal(d_row[:], dinv[:])
            dbc_ps = smps.tile([128, 2], F32, tag="sm", name="dbcps")
            nc.tensor.matmul(dbc_ps[:], ones_row[:], d_row[:],
                             start=True, stop=True)
            dbc = gxp.tile([128, 1], F32, tag="gx4", name="dbc")
            nc.vector.tensor_copy(dbc[:], dbc_ps[:, 0:1])
            ss = gxp.tile([128, IC], F32, tag="gx4", name="ss")
            nc.vector.scalar_tensor_tensor(ss[:], gx[:].bitcast(F32), dbc[:],
                                           grng[:], OP.mult, OP.mult)
            nc.vector.tensor_scalar(ss[:], ss[:], 1.0, None, OP.add)
            ss_t[bb] = ss

        def emit_w2scale(bb, iis):
            # w2 * ss, alternating Pool/DVE so mm2 is fed quickly;
            # b0 into a fresh buffer, b1 in place
            ss = ss_t[bb]
            if bb == 0 and w2x[0] is None:
                w2x[0] = w2sp.tile([128, IC, C], F16, tag="w2s", name="w2s")
            dst = w2x[0] if bb == 0 else w2t
            for ii in iis:
                if ii % 2 == 0:
                    nc.vector.tensor_scalar(dst[:, ii, :], w2t[:, ii, :],
                                            ss[:, ii:ii + 1], None, OP.mult)
                else:
                    nc.scalar.activation(dst[:, ii, :], w2t[:, ii, :],
                                         AF.Copy, scale=ss[:, ii:ii + 1])
            if bb == 1:
                w2x[1] = w2t

        def emit_xres_dma(g):
            bb, tt = divmod(g, TC)
            xrs = []
            for ci in range(CC):
                xr = xrp.tile([128, TN], F32, tag="xr", name="xr")
                nc.sync.dma_start(
                    xr[:], xv[bb, :, ci, tt * TN:(tt + 1) * TN].bitcast(F32))
                xrs.append(xr)
            xr_t[g] = xrs

        def emit_mm2(g):
            bb, tt = divmod(g, TC)
            h = h_t[g]
            w2 = w2x[bb]
            for ci in range(CC):
                po = mmps.tile([128, TN], F32, tag="mm", name="po")
                csl = slice(ci * 128, (ci + 1) * 128)
                for ii in range(IC):
                    nc.tensor.matmul(po[:], w2[:, ii, csl], h[:, ii, :],
                                     start=(ii == 0), stop=(ii == IC - 1))
                # drain per ci: (po + b2p) + x_res fused on DVE
                o_sb = obp.tile([128, TN], F32, tag="ob", name="osb")
                nc.vector.scalar_tensor_tensor(
                    o_sb[:], po[:], b2p[:, ci:ci + 1], xr_t[g][ci][:],
                    OP.add, OP.add)
                nc.sync.dma_start(ov[bb][:, ci, tt * TN:(tt + 1) * TN],
                                  o_sb[:])

        # ---------------- emission schedule ----------------
        w1pT = singles.tile([128, CC, I], F32R)
        w1v = w1pT_d.rearrange("(cc p) i -> p cc i", p=128)
        for _ci in range(CC):
            nc.sync.dma_start(dwdg[:, _ci], dwdg_d[:, _ci])
        emit_x_dma(0)
        for _ci in range(CC):
            nc.sync.dma_start(w1pT[:, _ci], w1v[:, _ci])
        emit_x_dma(1)
        b1p = singles.tile([128, IC], F32)
        nc.sync.dma_start(b1p[:], b1p_d)
        b2p = singles.tile([128, CC], F32)
        nc.sync.dma_start(b2p[:], b2p_d)
        grng = singles.tile([128, IC], F32)
        nc.sync.dma_start(grng[:], grng_d)
        w2t = singles.tile([128, IC, C], F16)
        nc.scalar.dma_start(w2t[:], w2T_d.rearrange("(ic p) c -> p ic c", p=128))

        onesf = singles.tile([128, 1], F32)
        nc.vector.memset(onesf[:], 1.0)
        ones_col = singles.tile([128, 1], F32R)   # stats lhsT (K=128, M=1)
        nc.vector.tensor_copy(ones_col[:], onesf[:])
        onesrf = singles.tile([1, 128], F32)
        nc.vector.memset(onesrf[:], 1.0)
        ones_row = singles.tile([1, 128], F32R)   # bcast lhsT (K=1, M=128)
        nc.vector.tensor_copy(ones_row[:], onesrf[:])
        ones_col_bf = singles.tile([128, 1], BF16)
        nc.vector.tensor_copy(ones_col_bf[:], onesf[:])
        magic_row = singles.tile([1, TN], U32)
        nc.vector.memset(magic_row[:], 0x5F3759DF)
        one_row = singles.tile([1, 1], U32)
        nc.vector.memset(one_row[:], 1)
        magic_col = singles.tile([128, IC], U32)
        nc.vector.memset(magic_col[:], 0x5F3759DF)
        one_col = singles.tile([128, 1], U32)
        nc.vector.memset(one_col[:], 1)

        gxpart[0] = gxp.tile([128, IC, TC], F32, tag="gxpart", name="gxp0")
        gxpart[1] = gxp.tile([128, IC, TC], F32, tag="gxpart", name="gxp1")

        # tile 0 prologue (PE idles here once)
        for _ci in range(CC):
            emit_conv(0, (_ci,))
        emit_stats_a(0)
        emit_sumy(0)
        emit_sumsq(0)
        emit_bbc(0)
        emit_abc(0)
        emit_conv(1, (0, 1))   # half of tile 1's conv ahead of block 0
        emit_stats_a(1)

        for g in range(G):
            # prefetch x two tiles ahead; second half of next tile's conv
            if g + 2 < G:
                emit_x_dma(g + 2)
            if g + 1 < G:
                emit_conv(g + 1, (2, 3))
            if g >= 3 and g - 3 < TC:          # x_res for b0's mm2 tiles
                emit_xres_dma(g - 3)

            gn = g + 1 if g + 1 < G else None

            def interject(ii, gn=gn, g=g):
                if gn is not None:
                    if ii == 2:
                        emit_sumy(gn)
                    elif ii == 4:
                        emit_sumsq(gn)
                    elif ii == 6:
                        emit_bbc(gn)
                    elif ii == 8:
                        emit_abc(gn)
                if g == 4 and ii in (3, 7, 11):    # w2s for b0, in chunks
                    emit_w2scale(0, range(ii - 3, ii + 1))

            emit_mm1(g, interject)

            if g == TC - 1:
                emit_grn(0)
            if TC <= g < 2 * TC:
                emit_mm2(g - TC)
            if g + 2 < G:
                emit_conv(g + 2, (0, 1))   # first half of conv, a block early
                emit_stats_a(g + 2)

        emit_grn(1)
        emit_w2scale(1, range(IC))
        for t in range(TC):
            emit_xres_dma(TC + t)
            emit_mm2(TC + t)


def _diag_taps(dw):
    # [128, CC, K, 128]: lhsT diag matrices so conv taps run on PE
    out = np.zeros((128, CC, K, 128), np.float32)
    for ci in range(CC):
        for p in range(128):
            out[p, ci, :, p] = dw[ci * 128 + p]
    return out


def _host_prep(inputs):
    w1 = inputs["w1"].astype(np.float64)
    ln_g = inputs["ln_g"].astype(np.float64)
    ln_b = inputs["ln_b"].astype(np.float64)
    w2 = inputs["w2"].astype(np.float64)
    w1p = w1 * ln_g[None, :]                         # [I, C]
    prep = {
        "w1pT": np.ascontiguousarray(w1p.T).astype(np.float32),
        "b1p": (inputs["b1"].astype(np.float64) + w1 @ ln_b)
               .astype(np.float32).reshape(IC, 128).T.copy(),
        "w2T": np.ascontiguousarray(w2.T).astype(np.float16),
        "b2p": (inputs["b2"].astype(np.float64)
                + w2 @ inputs["grn_b"].astype(np.float64))
               .astype(np.float32).reshape(CC, 128).T.copy(),
        "grng": inputs["grn_g"].reshape(IC, 128).T.copy().astype(np.float32),
        "dww": inputs["dw_w"].reshape(C, K).reshape(CC, 128, K)
               .transpose(1, 0, 2).copy().astype(np.float32),
        "dwdg": _diag_taps(inputs["dw_w"].reshape(C, K)),
        "dwb": inputs["dw_b"].reshape(CC, 128).T.copy().astype(np.float32),
    }
    return prep


def run(inputs, trace=False, **kw):
    if "nc" not in _CACHE:
        _CACHE["nc"] = _build()
    nc = _CACHE["nc"]
    prep = _host_prep(inputs)
    x = np.asarray(inputs["x"], dtype=np.float32)
    in_maps = []
    for c in range(NCORES):
        m = dict(prep)
        m["x"] = np.ascontiguousarray(x[c * BPC:(c + 1) * BPC])
        in_maps.append(m)
    res = run_bass_kernel_spmd(nc, in_maps, core_ids=list(range(NCORES)),
                               trace=trace, **kw)
    out = np.concatenate([r["out"] for r in res.results], axis=0)
    return out, res


def kernel(**inputs):
    out, _ = run(inputs)
    return out
